# revision 24
# baseline (speedup 1.0000x reference)
"""Multi-head causal attention on 8 TRN2 NeuronCores.

Problem: B=4, S=2048, D=768, H=12 heads (dk=64), causal mask, f32.

Sharding: 8 cores = 4 batches x 2 head-groups (6 heads each).
Core c handles batch c//2 and heads [6*(c%2), 6*(c%2)+6).
Each core computes its partial output projection (over its 384 local
features); the pair-sum (the tensor-parallel "all-reduce after Wo") and
the bo bias add happen at unshard time on the host.

Per-core kernel (all layouts chosen so no on-device transposes needed):
  xt  [768,2048]  = x[b].T          (compute dtype)
  wq/wk/wv [768,384] = W[hslice].T  (columns = local head features)
  wo  [384,768]   = Wo[:, fslice].T
  qT = wq.T @ xt   [384,2048]  (dk-major rows; also kT)
  v  = xt.T @ wv   [2048,384]  (natural layout, augmented with a ones
                                column per head for the softmax denom)
  per head h, per 512-wide i-block (flash-style, no max subtraction --
  scores ~ N(0,1) so exp never overflows):
    scoresT[j,i] = kT_h[:,jtile].T @ qT_h[:,iblock]   (PSUM)
    diag tiles: += causal additive mask (-1e9 above diagonal)
    expT = exp(scoresT/8)                            (ScalarE, cast)
    ctxT_unnorm[e,i], l[i] = [v_h|1].T @ expT        (PSUM accumulate)
    ctxT[e,i] = ctxT_unnorm * (1/l) broadcast        (PE bcast + DVE)
  out_partial = ctxT.T @ wo  [2048,768] f32
"""

import os
import numpy as np
import ml_dtypes

import concourse.bass as bass
import concourse.tile as tile
import concourse.mybir as mybir
from concourse import bacc
from concourse.masks import make_identity

B, S, D, H = 4, 2048, 768, 12
DK, P = 64, 128
HL = H // 2            # 6 heads per core
DL = HL * DK           # 384 local features
KD = D // P            # 6 contraction chunks over d
MT = DL // P           # 3 row-tiles of qT/kT/ctxT
ST = S // P            # 16 s-tiles
NBLK = 1024            # i-block width
NB = S // NBLK         # 2 i-blocks
NEG = -1e9

CDT = mybir.dt.bfloat16
NP_CDT = ml_dtypes.bfloat16
F32 = mybir.dt.float32

N_CORES = 8


def _seg_bounds(c0, width):
    """Split [c0, width) into <=512-wide chunks aligned to 512 boundaries
    (each chunk stays within one PSUM bank)."""
    segs = []
    s = c0
    while s < width:
        e = min((s // 512 + 1) * 512, width)
        segs.append((s, e))
        s = e
    return segs


def _emit(nc, tc, xt_d, wq_d, wk_d, wv_d, wo_d, out_d):
    Exp = mybir.ActivationFunctionType.Exp

    with tc.tile_pool(name="persist", bufs=1) as per, \
         tc.tile_pool(name="ps", bufs=2, space="PSUM") as pp, \
         tc.tile_pool(name="pc", bufs=2, space="PSUM") as cp, \
         tc.tile_pool(name="sb_e", bufs=4) as ebp, \
         tc.tile_pool(name="sb_rl", bufs=2) as rlp, \
         tc.tile_pool(name="sb_o", bufs=3) as ob:
        xt = per.tile([P, KD, S], CDT)
        wq = per.tile([P, KD, DL], CDT)
        wk = per.tile([P, KD, DL], CDT)
        wv = per.tile([P, KD, DL], CDT)
        wo = per.tile([P, MT, D], CDT)
        qt = per.tile([P, MT, S], CDT)
        kt = per.tile([P, MT, S], CDT)
        v = per.tile([P, ST, HL, DK + 1], CDT)
        ctxt = per.tile([P, MT, S], CDT)
        tri = per.tile([P, P], CDT)

        for k in range(KD):
            nc.sync.dma_start(out=xt[:, k, :], in_=xt_d[k * P:(k + 1) * P, :])
            nc.sync.dma_start(out=wv[:, k, :], in_=wv_d[k * P:(k + 1) * P, :])
            nc.sync.dma_start(out=wq[:, k, :], in_=wq_d[k * P:(k + 1) * P, :])
            nc.sync.dma_start(out=wk[:, k, :], in_=wk_d[k * P:(k + 1) * P, :])
        for m in range(MT):
            nc.sync.dma_start(out=wo[:, m, :], in_=wo_d[m * P:(m + 1) * P, :])

        nc.vector.memset(v[:, :, :, DK], 1.0)
        # multiplicative causal mask for diagonal tiles of exp(scoresT)
        # [j(part), i(free)]: keep (1) where i >= j, else 0.
        nc.gpsimd.memset(tri, 1.0)
        nc.gpsimd.affine_select(
            out=tri, in_=tri, compare_op=mybir.AluOpType.is_ge,
            fill=0.0, base=0, pattern=[[1, P]], channel_multiplier=-1)

        # v projection (natural [s, e] layout, ones column appended per head)
        for st in range(ST):
            ps = pp.tile([P, NBLK], F32, tag="ps")
            for k in range(KD):
                nc.tensor.matmul(
                    ps[:, 0:DL], lhsT=xt[:, k, st * P:(st + 1) * P],
                    rhs=wv[:, k, :], start=(k == 0), stop=(k == KD - 1))
            nc.vector.tensor_copy(v[:, st, :, 0:DK], ps[:, 0:DL])

        # per head-pair: q/k projections then attention (interleaves PE-dense
        # projection work of pair m+1 under ACT-bound attention of pair m)
        for mh in range(MT):
            for (wt, dst) in ((wq, qt), (wk, kt)):
                for nb in range(S // NBLK):
                    ps = pp.tile([P, NBLK], F32, tag="ps")
                    for s0, s1 in _seg_bounds(0, NBLK):
                        for k in range(KD):
                            nc.tensor.matmul(
                                ps[:, s0:s1], lhsT=wt[:, k, mh * P:(mh + 1) * P],
                                rhs=xt[:, k, nb * NBLK + s0:nb * NBLK + s1],
                                start=(k == 0), stop=(k == KD - 1))
                    nc.vector.tensor_copy(
                        dst[:, mh, nb * NBLK:(nb + 1) * NBLK], ps)

            heads = (2 * mh, 2 * mh + 1)
            for ib in range(NB):
                i0 = ib * NBLK
                njt = (ib + 1) * (NBLK // P)
                pctxs = {h: cp.tile([DK + 1, NBLK], F32, tag="pc",
                                    name=f"pctx_h{h}_ib{ib}")
                         for h in heads}
                for jt in range(njt):
                    c0 = max(0, jt * P - i0)
                    for h in heads:
                        oh = (h % 2) * DK
                        pctx = pctxs[h]
                        ps = pp.tile([P, NBLK], F32, tag="ps")
                        for s0, s1 in _seg_bounds(c0, NBLK):
                            nc.tensor.matmul(
                                ps[:, s0:s1],
                                lhsT=kt[oh:oh + DK, mh, jt * P:(jt + 1) * P],
                                rhs=qt[oh:oh + DK, mh, i0 + s0:i0 + s1],
                                start=True, stop=True)
                        et = ebp.tile([P, NBLK], CDT, tag="et")
                        nc.scalar.activation(
                            et[:, c0:NBLK], ps[:, c0:NBLK], Exp, scale=0.125)
                        if jt * P >= i0:  # diagonal tile: zero i < j entries
                            nc.gpsimd.tensor_mul(
                                et[:, c0:c0 + P], et[:, c0:c0 + P], tri)
                        for s0, s1 in _seg_bounds(c0, NBLK):
                            # last jt whose causal range still reaches this
                            # 512-col PSUM bank: close the bank's group there
                            last_jt = min(njt, (i0 + (s0 // 512 + 1) * 512) // P) - 1
                            nc.tensor.matmul(
                                pctx[:, s0:s1], lhsT=v[:, jt, h, :],
                                rhs=et[:, s0:s1],
                                start=(jt == 0), stop=(jt == last_jt))
                # epilogue: copy out unnormalized ctx + denominator row first
                # (frees the PSUM tile), then normalize ctxt in place.
                for h in heads:
                    oh = (h % 2) * DK
                    pctx = pctxs[h]
                    csl = ctxt[oh:oh + DK, mh, i0:i0 + NBLK]
                    nc.vector.tensor_copy(csl, pctx[0:DK, :])
                    lsb = rlp.tile([1, NBLK], F32, tag="lsb")
                    nc.vector.tensor_copy(lsb, pctx[DK:DK + 1, :])
                    rl1 = rlp.tile([1, NBLK], F32, tag="rl1")
                    nc.vector.reciprocal_approx_fast(rl1, lsb)
                    rlb = rlp.tile([P, NBLK], F32, tag="rlb")
                    nc.gpsimd.partition_broadcast(rlb, rl1, channels=P)
                    nc.vector.tensor_mul(csl, csl, rlb[oh:oh + DK, :])

        # ---------------- output projection ----------------
        for st in range(ST):
            po = cp.tile([P, D], F32, tag="pc")
            for n0 in range(0, D, 512):
                nn = min(512, D - n0)
                for m in range(MT):
                    nc.tensor.matmul(
                        po[:, n0:n0 + nn],
                        lhsT=ctxt[:, m, st * P:(st + 1) * P],
                        rhs=wo[:, m, n0:n0 + nn],
                        start=(m == 0), stop=(m == MT - 1))
            osb = ob.tile([P, D], F32, tag="osb")
            if st % 2 == 0:
                nc.scalar.copy(osb, po)
            else:
                nc.vector.tensor_copy(osb, po)
            nc.sync.dma_start(out=out_d[st * P:(st + 1) * P, :], in_=osb)


def build_nc():
    nc = bacc.Bacc(trn_type="TRN2", target_bir_lowering=False, debug=False)
    xt_d = nc.dram_tensor("xt", [D, S], CDT, kind="ExternalInput").ap()
    wq_d = nc.dram_tensor("wq", [D, DL], CDT, kind="ExternalInput").ap()
    wk_d = nc.dram_tensor("wk", [D, DL], CDT, kind="ExternalInput").ap()
    wv_d = nc.dram_tensor("wv", [D, DL], CDT, kind="ExternalInput").ap()
    wo_d = nc.dram_tensor("wo", [DL, D], CDT, kind="ExternalInput").ap()
    out_d = nc.dram_tensor("out", [S, D], F32, kind="ExternalOutput").ap()
    with tile.TileContext(nc) as tc:
        _emit(nc, tc, xt_d, wq_d, wk_d, wv_d, wo_d, out_d)
    nc.compile()
    return nc


def make_in_maps(x, Wq, Wk, Wv, Wo):
    in_maps = []
    for c in range(N_CORES):
        b, g = c // 2, c % 2
        hsl = slice(g * DL, (g + 1) * DL)
        in_maps.append({
            "xt": np.ascontiguousarray(x[b].T).astype(NP_CDT),
            "wq": np.ascontiguousarray(Wq[hsl, :].T).astype(NP_CDT),
            "wk": np.ascontiguousarray(Wk[hsl, :].T).astype(NP_CDT),
            "wv": np.ascontiguousarray(Wv[hsl, :].T).astype(NP_CDT),
            "wo": np.ascontiguousarray(Wo[:, hsl].T).astype(NP_CDT),
        })
    return in_maps


_BUILT = None
LAST_RESULT = None


def _install_ntff_hook():
    """Provide the antenv.axon_hooks module run_bass_kernel_spmd expects
    for NTFF profiling under axon (the agent image ships only a stub
    antenv package)."""
    import sys
    import types
    if "antenv.axon_hooks" in sys.modules:
        return
    mod = types.ModuleType("antenv.axon_hooks")
    mod._hook = None

    def set_axon_ntff_profile_hook(h):
        mod._hook = h

    def get_axon_ntff_profile_hook():
        return mod._hook

    mod.set_axon_ntff_profile_hook = set_axon_ntff_profile_hook
    mod.get_axon_ntff_profile_hook = get_axon_ntff_profile_hook
    sys.modules["antenv.axon_hooks"] = mod
    import antenv
    antenv.axon_hooks = mod
    try:
        from trn_agent_boot.trn_boot import _ntff_profile_via_ctypes
        hook = _ntff_profile_via_ctypes("/opt/axon/libaxon_pjrt.so")
        if hook is not None:
            mod._hook = hook
    except Exception:
        pass


def kernel(**inputs):
    global _BUILT, LAST_RESULT
    from concourse.bass_utils import run_bass_kernel_spmd

    x = np.asarray(inputs["x"], np.float32)
    Wq = np.asarray(inputs["Wq"], np.float32)
    Wk = np.asarray(inputs["Wk"], np.float32)
    Wv = np.asarray(inputs["Wv"], np.float32)
    Wo = np.asarray(inputs["Wo"], np.float32)
    bo = np.asarray(inputs["bo"], np.float32)

    if _BUILT is None:
        _BUILT = build_nc()
    nc = _BUILT

    trace = bool(int(os.environ.get("KTRACE", "0")))
    if trace:
        _install_ntff_hook()
    in_maps = make_in_maps(x, Wq, Wk, Wv, Wo)
    res = run_bass_kernel_spmd(
        nc, in_maps, core_ids=list(range(N_CORES)), trace=trace)
    LAST_RESULT = res

    out = np.empty((B, S, D), np.float32)
    for b in range(B):
        out[b] = res.results[2 * b]["out"] + res.results[2 * b + 1]["out"]
    out += bo
    return out


# revision 27
# speedup vs baseline: 1.1250x; 1.1250x over previous
"""Multi-head causal attention on 8 TRN2 NeuronCores.

Problem: B=4, S=2048, D=768, H=12 heads (dk=64), causal mask, f32.

Sharding: 8 cores = 4 batches x 2 head-groups (6 heads each).
Core c handles batch c//2 and heads [6*(c%2), 6*(c%2)+6).
Each core computes its partial output projection (over its 384 local
features); the pair-sum (the tensor-parallel "all-reduce after Wo") and
the bo bias add happen at unshard time on the host.

Per-core kernel (all layouts chosen so no on-device transposes needed):
  xt  [768,2048]  = x[b].T          (compute dtype)
  wq/wk/wv [768,384] = W[hslice].T  (columns = local head features)
  wo  [384,768]   = Wo[:, fslice].T
  qT = wq.T @ xt   [384,2048]  (dk-major rows; also kT)
  v  = xt.T @ wv   [2048,384]  (natural layout, augmented with a ones
                                column per head for the softmax denom)
  per head h, per 512-wide i-block (flash-style, no max subtraction --
  scores ~ N(0,1) so exp never overflows):
    scoresT[j,i] = kT_h[:,jtile].T @ qT_h[:,iblock]   (PSUM)
    diag tiles: += causal additive mask (-1e9 above diagonal)
    expT = exp(scoresT/8)                            (ScalarE, cast)
    ctxT_unnorm[e,i], l[i] = [v_h|1].T @ expT        (PSUM accumulate)
    ctxT[e,i] = ctxT_unnorm * (1/l) broadcast        (PE bcast + DVE)
  out_partial = ctxT.T @ wo  [2048,768] f32
"""

import os
import numpy as np
import ml_dtypes

import concourse.bass as bass
import concourse.tile as tile
import concourse.mybir as mybir
from concourse import bacc
from concourse.masks import make_identity

B, S, D, H = 4, 2048, 768, 12
DK, P = 64, 128
HL = H // 2            # 6 heads per core
DL = HL * DK           # 384 local features
KD = D // P            # 6 contraction chunks over d
MT = DL // P           # 3 row-tiles of qT/kT/ctxT
ST = S // P            # 16 s-tiles
NBLK = 1024            # i-block width
NB = S // NBLK         # 2 i-blocks
NEG = -1e9

CDT = mybir.dt.bfloat16
NP_CDT = ml_dtypes.bfloat16
F32 = mybir.dt.float32

N_CORES = 8


def _seg_bounds(c0, width):
    """Split [c0, width) into <=512-wide chunks aligned to 512 boundaries
    (each chunk stays within one PSUM bank)."""
    segs = []
    s = c0
    while s < width:
        e = min((s // 512 + 1) * 512, width)
        segs.append((s, e))
        s = e
    return segs


def _emit(nc, tc, xt_d, wq_d, wk_d, wv_d, wo_d, out_d):
    Exp = mybir.ActivationFunctionType.Exp

    with tc.tile_pool(name="persist", bufs=1) as per, \
         tc.tile_pool(name="ps", bufs=2, space="PSUM") as pp, \
         tc.tile_pool(name="pc", bufs=2, space="PSUM") as cp, \
         tc.tile_pool(name="sb_e", bufs=4) as ebp, \
         tc.tile_pool(name="sb_rl", bufs=2) as rlp, \
         tc.tile_pool(name="sb_o", bufs=3) as ob:
        xt = per.tile([P, KD, S], CDT)
        wq = per.tile([P, KD, DL], CDT)
        wk = per.tile([P, KD, DL], CDT)
        wv = per.tile([P, KD, DL], CDT)
        wo = per.tile([P, MT, D], CDT)
        qt = per.tile([P, MT, S], CDT)
        kt = per.tile([P, MT, S], CDT)
        v = per.tile([P, ST, HL, DK + 1], CDT)
        ctxt = per.tile([P, MT, S], CDT)
        ident = per.tile([P, P], CDT)
        maskm = per.tile([P, P], CDT)

        for k in range(KD):
            nc.sync.dma_start(out=xt[:, k, :], in_=xt_d[k * P:(k + 1) * P, :])
            nc.sync.dma_start(out=wv[:, k, :], in_=wv_d[k * P:(k + 1) * P, :])
            nc.sync.dma_start(out=wq[:, k, :], in_=wq_d[k * P:(k + 1) * P, :])
            nc.sync.dma_start(out=wk[:, k, :], in_=wk_d[k * P:(k + 1) * P, :])
        for m in range(MT):
            nc.sync.dma_start(out=wo[:, m, :], in_=wo_d[m * P:(m + 1) * P, :])

        nc.vector.memset(v[:, :, :, DK], 1.0)
        # Causal mask for diagonal tiles is injected on the PE itself:
        # matmul(lhsT=ident, rhs=maskm) writes maskm into the PSUM region as
        # the accumulation-group opener, and the k.q matmul accumulates on
        # top. maskm[k, i] = NEG where i < k (invalid), 0 where i >= k.
        make_identity(nc, ident)
        nc.gpsimd.memset(maskm, 0.0)
        nc.gpsimd.affine_select(
            out=maskm, in_=maskm, compare_op=mybir.AluOpType.is_ge,
            fill=NEG, base=0, pattern=[[1, P]], channel_multiplier=-1)

        # v projection (natural [s, e] layout, ones column appended per head)
        for st in range(ST):
            ps = pp.tile([P, NBLK], F32, tag="ps")
            for k in range(KD):
                nc.tensor.matmul(
                    ps[:, 0:DL], lhsT=xt[:, k, st * P:(st + 1) * P],
                    rhs=wv[:, k, :], start=(k == 0), stop=(k == KD - 1))
            nc.vector.tensor_copy(v[:, st, :, 0:DK], ps[:, 0:DL])

        # per head-pair: q/k projections then attention (interleaves PE-dense
        # projection work of pair m+1 under ACT-bound attention of pair m)
        for mh in range(MT):
            for (wt, dst) in ((wq, qt), (wk, kt)):
                for nb in range(S // NBLK):
                    ps = pp.tile([P, NBLK], F32, tag="ps")
                    for s0, s1 in _seg_bounds(0, NBLK):
                        for k in range(KD):
                            nc.tensor.matmul(
                                ps[:, s0:s1], lhsT=wt[:, k, mh * P:(mh + 1) * P],
                                rhs=xt[:, k, nb * NBLK + s0:nb * NBLK + s1],
                                start=(k == 0), stop=(k == KD - 1))
                    nc.vector.tensor_copy(
                        dst[:, mh, nb * NBLK:(nb + 1) * NBLK], ps)

            heads = (2 * mh, 2 * mh + 1)
            for ib in range(NB):
                i0 = ib * NBLK
                njt = (ib + 1) * (NBLK // P)
                pctxs = {h: cp.tile([DK + 1, NBLK], F32, tag="pc",
                                    name=f"pctx_h{h}_ib{ib}")
                         for h in heads}
                for jt in range(njt):
                    c0 = max(0, jt * P - i0)
                    diag = jt * P >= i0
                    for h in heads:
                        oh = (h % 2) * DK
                        pctx = pctxs[h]
                        ps = pp.tile([P, NBLK], F32, tag="ps")
                        if diag:
                            nc.tensor.matmul(
                                ps[:, c0:c0 + P], lhsT=ident, rhs=maskm,
                                start=True, stop=False)
                            nc.tensor.matmul(
                                ps[:, c0:c0 + P],
                                lhsT=kt[oh:oh + DK, mh, jt * P:(jt + 1) * P],
                                rhs=qt[oh:oh + DK, mh, i0 + c0:i0 + c0 + P],
                                start=False, stop=True)
                        for s0, s1 in _seg_bounds(c0 + P if diag else c0, NBLK):
                            nc.tensor.matmul(
                                ps[:, s0:s1],
                                lhsT=kt[oh:oh + DK, mh, jt * P:(jt + 1) * P],
                                rhs=qt[oh:oh + DK, mh, i0 + s0:i0 + s1],
                                start=True, stop=True)
                        et = ebp.tile([P, NBLK], CDT, tag="et")
                        nc.scalar.activation(
                            et[:, c0:NBLK], ps[:, c0:NBLK], Exp, scale=0.125)
                        for s0, s1 in _seg_bounds(c0, NBLK):
                            # last jt whose causal range still reaches this
                            # 512-col PSUM bank: close the bank's group there
                            last_jt = min(njt, (i0 + (s0 // 512 + 1) * 512) // P) - 1
                            nc.tensor.matmul(
                                pctx[:, s0:s1], lhsT=v[:, jt, h, :],
                                rhs=et[:, s0:s1],
                                start=(jt == 0), stop=(jt == last_jt))
                # epilogue: copy out unnormalized ctx + denominator row first
                # (frees the PSUM tile), then normalize ctxt in place.
                for h in heads:
                    oh = (h % 2) * DK
                    pctx = pctxs[h]
                    csl = ctxt[oh:oh + DK, mh, i0:i0 + NBLK]
                    nc.vector.tensor_copy(csl, pctx[0:DK, :])
                    lsb = rlp.tile([1, NBLK], F32, tag="lsb")
                    nc.vector.tensor_copy(lsb, pctx[DK:DK + 1, :])
                    rl1 = rlp.tile([1, NBLK], F32, tag="rl1")
                    nc.vector.reciprocal_approx_fast(rl1, lsb)
                    rlb = rlp.tile([P, NBLK], F32, tag="rlb")
                    nc.gpsimd.partition_broadcast(rlb, rl1, channels=P)
                    nc.vector.tensor_mul(csl, csl, rlb[oh:oh + DK, :])

        # ---------------- output projection ----------------
        for st in range(ST):
            po = cp.tile([P, D], F32, tag="pc")
            for n0 in range(0, D, 512):
                nn = min(512, D - n0)
                for m in range(MT):
                    nc.tensor.matmul(
                        po[:, n0:n0 + nn],
                        lhsT=ctxt[:, m, st * P:(st + 1) * P],
                        rhs=wo[:, m, n0:n0 + nn],
                        start=(m == 0), stop=(m == MT - 1))
            osb = ob.tile([P, D], F32, tag="osb")
            if st % 2 == 0:
                nc.scalar.copy(osb, po)
            else:
                nc.vector.tensor_copy(osb, po)
            nc.sync.dma_start(out=out_d[st * P:(st + 1) * P, :], in_=osb)


def build_nc():
    nc = bacc.Bacc(trn_type="TRN2", target_bir_lowering=False, debug=False)
    xt_d = nc.dram_tensor("xt", [D, S], CDT, kind="ExternalInput").ap()
    wq_d = nc.dram_tensor("wq", [D, DL], CDT, kind="ExternalInput").ap()
    wk_d = nc.dram_tensor("wk", [D, DL], CDT, kind="ExternalInput").ap()
    wv_d = nc.dram_tensor("wv", [D, DL], CDT, kind="ExternalInput").ap()
    wo_d = nc.dram_tensor("wo", [DL, D], CDT, kind="ExternalInput").ap()
    out_d = nc.dram_tensor("out", [S, D], F32, kind="ExternalOutput").ap()
    with tile.TileContext(nc) as tc:
        _emit(nc, tc, xt_d, wq_d, wk_d, wv_d, wo_d, out_d)
    nc.compile()
    return nc


def make_in_maps(x, Wq, Wk, Wv, Wo):
    in_maps = []
    for c in range(N_CORES):
        b, g = c // 2, c % 2
        hsl = slice(g * DL, (g + 1) * DL)
        in_maps.append({
            "xt": np.ascontiguousarray(x[b].T).astype(NP_CDT),
            "wq": np.ascontiguousarray(Wq[hsl, :].T).astype(NP_CDT),
            "wk": np.ascontiguousarray(Wk[hsl, :].T).astype(NP_CDT),
            "wv": np.ascontiguousarray(Wv[hsl, :].T).astype(NP_CDT),
            "wo": np.ascontiguousarray(Wo[:, hsl].T).astype(NP_CDT),
        })
    return in_maps


_BUILT = None
LAST_RESULT = None


def _install_ntff_hook():
    """Provide the antenv.axon_hooks module run_bass_kernel_spmd expects
    for NTFF profiling under axon (the agent image ships only a stub
    antenv package)."""
    import sys
    import types
    if "antenv.axon_hooks" in sys.modules:
        return
    mod = types.ModuleType("antenv.axon_hooks")
    mod._hook = None

    def set_axon_ntff_profile_hook(h):
        mod._hook = h

    def get_axon_ntff_profile_hook():
        return mod._hook

    mod.set_axon_ntff_profile_hook = set_axon_ntff_profile_hook
    mod.get_axon_ntff_profile_hook = get_axon_ntff_profile_hook
    sys.modules["antenv.axon_hooks"] = mod
    import antenv
    antenv.axon_hooks = mod
    try:
        from trn_agent_boot.trn_boot import _ntff_profile_via_ctypes
        hook = _ntff_profile_via_ctypes("/opt/axon/libaxon_pjrt.so")
        if hook is not None:
            mod._hook = hook
    except Exception:
        pass


def kernel(**inputs):
    global _BUILT, LAST_RESULT
    from concourse.bass_utils import run_bass_kernel_spmd

    x = np.asarray(inputs["x"], np.float32)
    Wq = np.asarray(inputs["Wq"], np.float32)
    Wk = np.asarray(inputs["Wk"], np.float32)
    Wv = np.asarray(inputs["Wv"], np.float32)
    Wo = np.asarray(inputs["Wo"], np.float32)
    bo = np.asarray(inputs["bo"], np.float32)

    if _BUILT is None:
        _BUILT = build_nc()
    nc = _BUILT

    trace = bool(int(os.environ.get("KTRACE", "0")))
    if trace:
        _install_ntff_hook()
    in_maps = make_in_maps(x, Wq, Wk, Wv, Wo)
    res = run_bass_kernel_spmd(
        nc, in_maps, core_ids=list(range(N_CORES)), trace=trace)
    LAST_RESULT = res

    out = np.empty((B, S, D), np.float32)
    for b in range(B):
        out[b] = res.results[2 * b]["out"] + res.results[2 * b + 1]["out"]
    out += bo
    return out
